# revision 6
# baseline (speedup 1.0000x reference)
"""Trainium2 Bass kernel for nn_DGN4 (gnn_message_passing).

Reference semantics (B=4, T=2048, D=256, K_SIM=8, K_CON=4):
  xn    = x / max(||x||, 1e-12)                       (row L2-normalize)
  sim   = xn @ xn^T, causally masked (strictly past), masked = -1e9
  A_sim = top-8 per row (one-hot), zeroed outside past
  A_con = "bottom-4" of sim excluding A_sim -- because masked/future
          entries score +1e9 in the negated space, the reference's con
          picks land on future columns (then zeroed by the causal mask)
          for every row with 2048 - t >= 4.  Only rows T-3..T-1 get
          1, 2, 3 real con-neighbors.
  msg_* = degree-normalized mean of selected x rows
  ctx   = alpha*msg_pos + (1-alpha)*msg_neg
  delta = gelu(mix*x + (1-mix)*ctx) * scale   (exact erf gelu)

Sharding: 8 cores = 4 batches x 2 row-shards.  One uniform SPMD program;
per-core differences are data only (odd cores get adjacent 128-row blocks
swapped -- an involution -- so static tile offsets address their rows).

Design (v2):
  - sim strips computed on the PE in f32r (1 cycle/row) straight into
    multi-bank PSUM tiles; never copied to SBUF.
  - causal keep-masks applied *additively* inside PSUM by one extra
    matmul per tile (identity weights x per-tile -1e9 mask matrix).
  - top-8 via DVE Max8 reading PSUM directly; threshold tau = v8[7].
  - adjacency indicator A01 = (sim >= tau) built by the *Pool* engine
    (idle otherwise) writing exact 0/1 bf16 to SBUF.
  - A01 block-transposed by the DMA XBAR (dma_start_transpose, one
    instruction per tile, 3-D out AP = batched 128x128 transposes) --
    no PE transposes, no PSUM->SBUF copies for the adjacency.
  - aggregation = bf16 matmuls (AT block x x_bf16 block) accumulated in
    a 1-bank PSUM ctx tile; coef = (1-mix)*alpha/deg folded in at blend.
  - contrarian branch only on the full-width tile (value-based bottom-4,
    no match_replace needed: top-8 can never be among the 8 smallest).
"""

import numpy as np

B, T, D = 4, 2048, 256
PB = 128                 # partition block
NBLK = T // PB           # 16 row/col blocks per batch
NTILE = 8                # program tiles per core
# width (in 128-blocks) and own-block index per program tile; widths pair to 18
WB = [2, 16, 4, 14, 6, 12, 8, 10]
OWN = [0, 15, 2, 13, 4, 11, 6, 9]
# strip-phase order: alternate small(<=1024) / big(>1024) PSUM pools,
# narrow-xnT-dependency tiles early, the con tile (k=1) mid-stream
ORDER = [6, 3, 4, 1, 2, 5, 0, 7]
NEG = -1.0e9
# per-tile adjacency-indicator route: "pool" / "dve2x" (Act copy + is_ge on
# that engine) or "dvepsum" (DVE is_ge straight from PSUM, no copy)
AB_ROUTE = {0: "dvepsum", 1: "pool", 2: "dvepsum", 3: "pool",
            4: "dvepsum", 5: "dve2x", 6: "dvepsum", 7: "dve2x"}

_PROGRAMS = {}


def _build_mnegs():
    """Additive -1e9 masks for the last two 128-col blocks of each tile.

    mnegs[parity] is [128, NTILE, 256] f32; 0 = keep, -1e9 = drop.
    Program col-block j holds actual block pi(j) (pi = identity / pair-swap).
    """
    tri_keep = (np.arange(PB)[None, :] < np.arange(PB)[:, None])
    out = []
    for parity in (0, 1):
        pi = (lambda j: j) if parity == 0 else (lambda j: j ^ 1)
        m = np.zeros((PB, NTILE, 2 * PB), np.float32)
        for k in range(NTILE):
            o_act = pi(OWN[k])
            for idx, j in enumerate((WB[k] - 2, WB[k] - 1)):
                a_act = pi(j)
                if a_act < o_act:
                    keep = np.ones((PB, PB), bool)
                elif a_act == o_act:
                    keep = tri_keep
                else:
                    keep = np.zeros((PB, PB), bool)
                m[:, k, idx * PB:(idx + 1) * PB] = np.where(keep, 0.0, NEG)
        out.append(m)
    return out


def _build_program(affine):
    import concourse.bacc as bacc
    import concourse.tile as tile
    from concourse import mybir

    f32 = mybir.dt.float32
    f32r = mybir.dt.float32r
    bf16 = mybir.dt.bfloat16
    Alu = mybir.AluOpType
    Act = mybir.ActivationFunctionType

    nc = bacc.Bacc(None)
    x_ext = nc.declare_dram_parameter("x", [T, D], f32, isOutput=False)
    mneg_ext = nc.declare_dram_parameter("mnegs", [PB, NTILE * 2 * PB], bf16, isOutput=False)
    consts_ext = nc.declare_dram_parameter("consts", [PB, 8], f32, isOutput=False)
    eye_ext = nc.declare_dram_parameter("eye", [PB, PB], f32, isOutput=False)
    eyeb_ext = nc.declare_dram_parameter("eyeb", [PB, PB], bf16, isOutput=False)
    gain_ext = nc.declare_dram_parameter("gain_bc", [PB, D], f32, isOutput=False)
    bias_ext = nc.declare_dram_parameter("bias_bc", [PB, D], f32, isOutput=False)
    out_ext = nc.declare_dram_parameter("out", [NTILE * PB, D], f32, isOutput=True)

    with tile.TileContext(nc) as tc:
        with (
            tc.tile_pool(name="singles", bufs=1) as singles,
            tc.tile_pool(name="scr", bufs=2) as scr,
            tc.tile_pool(name="a01p", bufs=2) as a01p,
            tc.tile_pool(name="wp", bufs=2) as wp,
            tc.tile_pool(name="atp", bufs=2) as atp,
            tc.tile_pool(name="small", bufs=4) as small,
            tc.tile_pool(name="bl", bufs=3) as blp,
            tc.tile_pool(name="conp", bufs=1) as conp,
            tc.tile_pool(name="ps_big", bufs=1, space="PSUM") as ps_big,
            tc.tile_pool(name="ps_sml", bufs=1, space="PSUM") as ps_sml,
            tc.tile_pool(name="ps_ctx", bufs=1, space="PSUM") as ps_ctx,
            tc.tile_pool(name="ps_aux", bufs=1, space="PSUM") as ps_aux,
        ):
            # ---- loads ----------------------------------------------------
            x_all = singles.tile([PB, NBLK, D], f32)
            x_re = x_ext[:].rearrange("(c p) d -> p c d", p=PB)
            for grp in range(4):
                nc.sync.dma_start(out=x_all[:, grp * 4:(grp + 1) * 4, :],
                                  in_=x_re[:, grp * 4:(grp + 1) * 4, :])
            consts_sb = singles.tile([PB, 8], f32)
            nc.sync.dma_start(out=consts_sb, in_=consts_ext[:])
            eye_sb = singles.tile([PB, PB], f32)
            nc.sync.dma_start(out=eye_sb, in_=eye_ext[:])
            eyeb_sb = singles.tile([PB, PB], bf16)
            nc.sync.dma_start(out=eyeb_sb, in_=eyeb_ext[:])
            mneg_sb = singles.tile([PB, NTILE, 2 * PB], bf16)
            nc.sync.dma_start(out=mneg_sb, in_=mneg_ext[:].rearrange("p (k m) -> p k m", k=NTILE))
            gain_sb = singles.tile([PB, D], f32)
            nc.sync.dma_start(out=gain_sb, in_=gain_ext[:])
            bias_sb = singles.tile([PB, D], f32)
            nc.sync.dma_start(out=bias_sb, in_=bias_ext[:])

            # first-touch: TS instructions encode a single sync wait, so no
            # TS op may be the first on its engine to see two DMA queues.
            touch_f = singles.tile([PB, 6], f32)
            nc.vector.tensor_copy(touch_f[:, 0:1], x_all[:, 0, 0:1])
            nc.vector.tensor_copy(touch_f[:, 1:2], consts_sb[:, 0:1])
            nc.vector.tensor_copy(touch_f[:, 2:3], gain_sb[:, 0:1])
            nc.vector.tensor_copy(touch_f[:, 3:4], bias_sb[:, 0:1])
            touch_b = singles.tile([PB, 2], bf16)
            nc.vector.tensor_copy(touch_b[:, 0:1], eyeb_sb[:, 0:1])
            nc.vector.tensor_copy(touch_b[:, 1:2], mneg_sb[:, 0, 0:1])
            touch_p = singles.tile([PB, 2], f32)
            nc.gpsimd.tensor_copy(touch_p[:, 0:1], x_all[:, 1, 0:1])
            nc.gpsimd.tensor_copy(touch_p[:, 1:2], consts_sb[:, 1:2])

            mix_ap = consts_sb[:, 0:1]
            c1_ap = consts_sb[:, 1:2]       # (1-mix)*alpha
            c2_ap = consts_sb[:, 2:3]       # (1-mix)*(1-alpha)
            scale_ap = consts_sb[:, 3:4]

            # ---- prologue: norms, normalize, bf16 cast --------------------
            nrm2 = singles.tile([PB, NBLK], f32)
            nrm = singles.tile([PB, NBLK], f32)
            rinv = singles.tile([PB, NBLK], f32)
            xn = singles.tile([PB, NBLK, D], f32)
            x_bf = singles.tile([PB, NBLK, D], bf16)
            xnT = singles.tile([PB, 2, T], f32r)
            for grp in range(4):
                g4 = slice(grp * 4, grp * 4 + 4)
                sq = scr.tile([PB, 4, D], f32, tag="sq")
                for u, c in enumerate(range(grp * 4, grp * 4 + 4)):
                    nc.gpsimd.tensor_mul(sq[:, u, :], x_all[:, c, :],
                                         x_all[:, c, :])
                nc.vector.tensor_reduce(nrm2[:, g4], sq,
                                        axis=mybir.AxisListType.X, op=Alu.add)
                nc.scalar.activation(nrm[:, g4], nrm2[:, g4], Act.Sqrt)
                nc.gpsimd.tensor_scalar_max(nrm[:, g4], nrm[:, g4], 1e-12)
                nc.vector.reciprocal(rinv[:, g4], nrm[:, g4])
                for c in range(grp * 4, grp * 4 + 4):
                    eng = nc.vector if c % 2 == 0 else nc.gpsimd
                    eng.tensor_scalar_mul(xn[:, c, :], x_all[:, c, :],
                                          rinv[:, c:c + 1])
                for c in range(grp * 4, grp * 4 + 4):
                    nc.gpsimd.tensor_copy(x_bf[:, c, :], x_all[:, c, :])

            # xnT via PE transposes staged through the 1-bank aux PSUM.
            # batch = 4 blocks x 1 half -> one [128,512] copy to SBUF.
            def emit_xnt_batch(cg, h):
                psT = ps_aux.tile([PB, 512], f32, tag="aux")
                for u in range(4):
                    c = cg * 4 + u
                    nc.tensor.transpose(psT[:, u * PB:(u + 1) * PB],
                                        xn[:, c, h * PB:(h + 1) * PB], eye_sb)
                nc.scalar.copy(xnT[:, h, cg * 512:(cg + 1) * 512], psT)

            # ---- per-tile phases ------------------------------------------
            state = {}

            def emit_strip_phase(k):
                nb, own = WB[k], OWN[k]
                W = nb * PB
                pool = ps_big if nb > 8 else ps_sml
                cap = 2048 if nb > 8 else 1024
                s_t = pool.tile([PB, cap], f32, tag="strip")
                n512 = (W + 511) // 512
                for j in range(n512):
                    lo = j * 512
                    n = min(512, W - lo)
                    last_region = (j == n512 - 1)
                    for h in (0, 1):
                        nc.tensor.matmul(
                            s_t[:, lo:lo + n],
                            xnT[:, h, own * PB:(own + 1) * PB],
                            xnT[:, h, lo:lo + n],
                            start=(h == 0), stop=(h == 1 and not last_region))
                # additive causal mask on the last two blocks; closes the
                # final accumulation region
                nc.tensor.matmul(s_t[:, W - 256:W], eyeb_sb, mneg_sb[:, k, :],
                                 start=False, stop=True)

                v8 = small.tile([PB, 8], f32, tag="v8")
                nc.vector.max(out=v8, in_=s_t[:, 0:W])
                tau = small.tile([PB, 1], f32, tag="tau")
                nc.vector.tensor_scalar_max(tau, v8[:, 7:8], -1e8)
                cnt8 = small.tile([PB, 8], f32, tag="cnt8")
                cnt = small.tile([PB, 1], f32, tag="cnt")
                nc.vector.tensor_scalar(cnt8, v8, -1e8, None, op0=Alu.is_gt,
                                        op1=Alu.add, accum_out=cnt)
                deg = small.tile([PB, 1], f32, tag="deg")
                nc.vector.tensor_scalar_max(deg, cnt, 1.0)
                rdeg = small.tile([PB, 1], f32, tag="rdeg")
                nc.vector.reciprocal(rdeg, deg)
                coef = small.tile([PB, 1], f32, tag="coef")
                nc.vector.tensor_scalar(coef, rdeg, c1_ap, None, op0=Alu.mult)

                A01 = a01p.tile([PB, T], bf16, tag="A")
                route = AB_ROUTE[k]
                if route == "dvepsum":
                    # threshold directly from PSUM on the DVE
                    nc.vector.tensor_scalar(A01[:, 0:W], s_t[:, 0:W], tau, None,
                                            op0=Alu.is_ge)
                    w_sb = None
                else:
                    # GPSIMD cannot read PSUM: drain the strip once on the
                    # Activation engine, threshold on Pool or DVE (2x mode)
                    w_sb = wp.tile([PB, T], f32, tag="w")
                    nc.scalar.copy(w_sb[:, 0:W], s_t[:, 0:W])
                    eng = nc.gpsimd if route == "pool" else nc.vector
                    eng.tensor_scalar(A01[:, 0:W], w_sb[:, 0:W], tau, None,
                                      op0=Alu.is_ge)

                con = None
                if k == 1:
                    negw = conp.tile([PB, T], f32, tag="negw")
                    nc.gpsimd.tensor_scalar_mul(negw, w_sb, -1.0)
                    vc8 = conp.tile([PB, 8], f32, tag="vc8")
                    nc.vector.max(out=vc8, in_=negw)
                    hi = conp.tile([PB, T], bf16, tag="hi")
                    nc.gpsimd.tensor_scalar(hi, w_sb, -1e8, None, op0=Alu.is_gt)
                    thrn = conp.tile([PB, 1], f32, tag="thrn")
                    nc.vector.tensor_scalar_mul(thrn, vc8[:, 3:4], -1.0)
                    lo_t = conp.tile([PB, T], bf16, tag="lo")
                    nc.vector.tensor_scalar(lo_t, w_sb, thrn, None, op0=Alu.is_le)
                    Ac01 = a01p.tile([PB, T], bf16, tag="A")
                    nc.gpsimd.tensor_mul(Ac01, lo_t, hi)
                    cntc4 = conp.tile([PB, 4], f32, tag="cntc4")
                    cntc = conp.tile([PB, 1], f32, tag="cntc")
                    nc.vector.tensor_scalar(cntc4, vc8[:, 0:4], 1e8, None,
                                            op0=Alu.is_lt, op1=Alu.add,
                                            accum_out=cntc)
                    degc = conp.tile([PB, 1], f32, tag="degc")
                    nc.vector.tensor_scalar_max(degc, cntc, 1.0)
                    rdegc = conp.tile([PB, 1], f32, tag="rdegc")
                    nc.vector.reciprocal(rdegc, degc)
                    coefc = conp.tile([PB, 1], f32, tag="coefc")
                    nc.vector.tensor_scalar(coefc, rdegc, c2_ap, None, op0=Alu.mult)
                    ATc = atp.tile([PB, NBLK, PB], bf16, tag="AT")
                    nc.sync.dma_start_transpose(ATc[:, 0:nb, :], Ac01[:, 0:W])
                    con = (ATc, coefc)

                AT = atp.tile([PB, NBLK, PB], bf16, tag="AT")
                nc.sync.dma_start_transpose(AT[:, 0:nb, :], A01[:, 0:W])
                state[k] = (AT, coef, con)

            def emit_agg_phase(k):
                nb, own = WB[k], OWN[k]
                AT, coef, con = state.pop(k)
                ctx = ps_ctx.tile([PB, D], f32, tag="ctx")
                for c in range(nb):
                    nc.tensor.matmul(ctx, AT[:, c, :], x_bf[:, c, :],
                                     start=(c == 0), stop=(c == nb - 1))
                xp = blp.tile([PB, D], f32, tag="xp")
                nc.gpsimd.tensor_scalar_mul(xp, x_all[:, own, :], mix_ap)
                bl = blp.tile([PB, D], f32, tag="bl")
                nc.vector.scalar_tensor_tensor(bl, ctx, coef, xp,
                                               op0=Alu.mult, op1=Alu.add)
                if con is not None:
                    ATc, coefc = con
                    ctxc = ps_aux.tile([PB, 512], f32, tag="aux")
                    for c in range(nb):
                        nc.tensor.matmul(ctxc[:, 0:D], ATc[:, c, :], x_bf[:, c, :],
                                         start=(c == 0), stop=(c == nb - 1))
                    bl2 = blp.tile([PB, D], f32, tag="bl")
                    nc.vector.scalar_tensor_tensor(bl2, ctxc[:, 0:D], coefc, bl,
                                                   op0=Alu.mult, op1=Alu.add)
                    bl = bl2
                if affine:
                    z = blp.tile([PB, D], f32, tag="z")
                    nc.gpsimd.tensor_mul(z, bl, gain_sb)
                    nc.gpsimd.tensor_add(z, z, bias_sb)
                    bl = z
                g_t = blp.tile([PB, D], f32, tag="g")
                nc.scalar.activation(g_t, bl, Act.Gelu)
                d_t = blp.tile([PB, D], f32, tag="d")
                nc.gpsimd.tensor_scalar_mul(d_t, g_t, scale_ap)
                nc.sync.dma_start(out=out_ext[k * PB:(k + 1) * PB, :], in_=d_t)

            # ---- schedule -------------------------------------------------
            # xnT batches for blocks 0..7, first tile, xnT 8..15, rest.
            # agg(k) emitted after strips(next) so the XBAR latency of tile k
            # is covered by the next tile's PE strip matmuls.
            for cg in (0, 1):
                for h in (0, 1):
                    emit_xnt_batch(cg, h)
            emit_strip_phase(ORDER[0])
            for cg in (2, 3):
                for h in (0, 1):
                    emit_xnt_batch(cg, h)
            for i in range(1, NTILE):
                emit_strip_phase(ORDER[i])
                emit_agg_phase(ORDER[i - 1])
            emit_agg_phase(ORDER[-1])

    nc.compile()
    return nc


def _get_program(affine=False):
    if affine not in _PROGRAMS:
        _PROGRAMS[affine] = _build_program(affine)
    return _PROGRAMS[affine]


def _host_prep(inputs):
    """Returns (affine, in_maps) replicating the per-core data layout."""
    import ml_dtypes
    bf = ml_dtypes.bfloat16

    x = np.ascontiguousarray(np.asarray(inputs["x"], dtype=np.float32))
    gain = np.asarray(inputs["gain"], dtype=np.float32).reshape(D)
    bias = np.asarray(inputs["bias"], dtype=np.float32).reshape(D)
    log_mix = float(np.asarray(inputs["log_mix"]))
    log_alpha = float(np.asarray(inputs["log_alpha"]))
    log_scale = float(np.asarray(inputs["log_scale"]))

    mix = np.float32(1.0 / (1.0 + np.exp(-np.float64(log_mix))))
    alpha = np.float32(1.0 / (1.0 + np.exp(-np.float64(log_alpha))))
    scale = np.float32(np.logaddexp(0.0, np.float64(log_scale)) + 0.01)
    affine = not (np.all(gain == 1.0) and np.all(bias == 0.0))

    consts = np.zeros((PB, 8), np.float32)
    consts[:, 0] = mix
    consts[:, 1] = (np.float32(1.0) - mix) * alpha
    consts[:, 2] = (np.float32(1.0) - mix) * (np.float32(1.0) - alpha)
    consts[:, 3] = scale
    gain_bc = np.ascontiguousarray(np.broadcast_to(gain[None, :], (PB, D)))
    bias_bc = np.ascontiguousarray(np.broadcast_to(bias[None, :], (PB, D)))
    eye = np.eye(PB, dtype=np.float32)
    eyeb = np.eye(PB, dtype=bf)
    mnegs = [m.reshape(PB, NTILE * 2 * PB).astype(bf) for m in _build_mnegs()]

    swap_perm = np.arange(NBLK).reshape(-1, 2)[:, ::-1].reshape(-1)

    in_maps = []
    for c in range(8):
        b, p = c // 2, c % 2
        xb = x[b]
        if p:
            xb = np.ascontiguousarray(
                xb.reshape(NBLK, PB, D)[swap_perm].reshape(T, D))
        in_maps.append({
            "x": xb,
            "mnegs": mnegs[p],
            "consts": consts,
            "eye": eye,
            "eyeb": eyeb,
            "gain_bc": gain_bc,
            "bias_bc": bias_bc,
        })
    return affine, in_maps


def kernel(**inputs):
    affine, in_maps = _host_prep(inputs)
    from concourse.bass_utils import run_bass_kernel_spmd
    nc = _get_program(affine)
    res = run_bass_kernel_spmd(nc, in_maps, list(range(8))).results

    out = np.empty((B, T, D), np.float32)
    for c in range(8):
        b, p = c // 2, c % 2
        o = np.asarray(res[c]["out"])
        for k in range(NTILE):
            g_act = OWN[k] ^ p
            out[b, g_act * PB:(g_act + 1) * PB, :] = o[k * PB:(k + 1) * PB, :]
    return out
